# revision 8
# baseline (speedup 1.0000x reference)
"""Multi-head attention Trainium2 Bass kernel (v2: bf16 + DP4xTP2).

Problem: B=4, S=2048, H=16, DH=64, D=1024, fp32 in/out.
  q/k/v = hidden @ W{q,k,v}.T + b; scores = q k^T / 8; probs = softmax;
  ctx = probs v; out = ctx @ Wo.T + bo.

Sharding: 4-way data-parallel over batch x 2-way tensor-parallel over
heads. Core c owns batch c//2 and heads (c%2)*8 .. +8 (a 512-wide
feature slice). Each core computes a partial output projection over its
512 context features; host sums the 2 partials per batch and adds bo.

All matmuls in bf16 (fp32r streams rows at ~2 cyc on HW, bf16 at 1).
Host pre-transposes hidden and weights so the device does ZERO PE
transposes:
  hT   [e(8x128p), tok 2048]   <- host hid[b].T
  qT/kT [feat(4x128p), tok]    = WT-lhsT x hT-rhs   (+bias, q pre-scaled)
  V    [tok(16x128p), feat 512] = hT-lhsT x WvT-rhs  (+bias) -> vaug
  vaug [tok 128p, kc 16, h 8, 65] (64 V cols + ones col for denominator)
  S    [kt 128p, q 1024] = kT-lhsT x qT-rhs  (PSUM, fp32)
  P    = exp(S) -> bf16 (ACT engine; no max subtraction: scores ~ N(0,1))
  ctx^T/den = vaug-lhsT x P-rhs accumulated over kt chunks (PSUM [65, q])
  ctxn = ctx^T * (1/den)  (recip on DVE, partition-broadcast on GpSimd)
  out_partial [tok, fo] = ctxn-lhsT x woT-rhs -> bf16 -> DRAM
"""
import numpy as np
import ml_dtypes

import concourse.bass as bass
import concourse.tile as tile
from concourse import bacc, mybir
from concourse import bass_utils

F32 = mybir.dt.float32
F32R = mybir.dt.float32r
BF16 = mybir.dt.bfloat16
I16 = mybir.dt.int16
EXP = mybir.ActivationFunctionType.Exp
BF = ml_dtypes.bfloat16

# Schraudolph exp in bf16 bit-space: bits = round(x*log2e*128 + 127*128 - 7)
SCH_MUL = 128.0 * 1.4426950408889634
SCH_BIAS = 127.0 * 128 - 7.0
OFFLOAD = {2, 5, 8, 11, 14}   # kt chunks whose exp runs on DVE instead of ACT

B = 4
S = 2048
D = 1024
NCORES = 8
P = 128          # partitions
F = 512          # per-core feature slice (8 heads x 64)
H = 8            # heads per core
EC = D // P      # 8 e-chunks (contraction for projections)
FC = F // P      # 4 feature chunks
KC = S // P      # 16 kt chunks
QB = S // 1024   # 2 q blocks


def build_nc():
    nc = bacc.Bacc("TRN2", target_bir_lowering=False, debug=False,
                   enable_asserts=True, num_devices=NCORES)

    hidT = nc.dram_tensor("hidT", [EC, P, S], BF16, kind="ExternalInput").ap()
    wqT = nc.dram_tensor("wqT", [EC, P, F], BF16, kind="ExternalInput").ap()
    wkT = nc.dram_tensor("wkT", [EC, P, F], BF16, kind="ExternalInput").ap()
    wvT = nc.dram_tensor("wvT", [EC, P, F], BF16, kind="ExternalInput").ap()
    woT = nc.dram_tensor("woT", [FC, P, D], BF16, kind="ExternalInput").ap()
    bq = nc.dram_tensor("bq", [P, FC], F32, kind="ExternalInput").ap()
    bk = nc.dram_tensor("bk", [P, FC], F32, kind="ExternalInput").ap()
    bv = nc.dram_tensor("bv", [1, F], F32, kind="ExternalInput").ap()
    out = nc.dram_tensor("out", [S, D], BF16, kind="ExternalOutput").ap()

    with tile.TileContext(nc) as tc:
        with (
            tc.tile_pool(name="const", bufs=1) as cpool,
            tc.tile_pool(name="wts", bufs=1) as wpool,
            tc.tile_pool(name="qkv", bufs=1) as qkv,
            tc.tile_pool(name="probs", bufs=4) as probsp,
            tc.tile_pool(name="recipp", bufs=2) as recipp,
            tc.tile_pool(name="recipbp", bufs=2) as recipbp,
            tc.tile_pool(name="ostage", bufs=4) as ostage,
            tc.tile_pool(name="psA", bufs=2, space="PSUM") as psA,
            tc.tile_pool(name="psC", bufs=2, space="PSUM") as psC,
        ):
            # ---- constants / weights ----
            bq_t = cpool.tile([P, FC], F32, tag="bq")
            bk_t = cpool.tile([P, FC], F32, tag="bk")
            bv_1 = cpool.tile([1, F], F32, tag="bv1")
            nc.sync.dma_start(bq_t[:], bq)
            nc.sync.dma_start(bk_t[:], bk)
            nc.sync.dma_start(bv_1[:], bv)
            # bv broadcast across partitions, viewed [P, h, 64]
            bv_b = cpool.tile([P, H, 64], F32, tag="bvb")
            nc.gpsimd.partition_broadcast(
                bv_b[:].rearrange("p h f -> p (h f)"), bv_1[:])
            # ones row (partition 64) for the denominator-broadcast matmul
            ones32 = cpool.tile([65, 64], F32, tag="ones32")
            nc.gpsimd.memset(ones32[:], 1.0)
            ones_r = cpool.tile([65, 64], F32R, tag="onesr")
            nc.vector.tensor_copy(ones_r[:], ones32[:])

            wq_s = wpool.tile([P, EC, F], BF16, tag="wq")
            wk_s = wpool.tile([P, EC, F], BF16, tag="wk")
            wv_s = wpool.tile([P, EC, F], BF16, tag="wv")
            wo_s = wpool.tile([P, FC, D], BF16, tag="wo")
            hT = wpool.tile([P, EC, S], BF16, tag="hT")
            nc.sync.dma_start(wq_s[:], wqT.rearrange("c p f -> p c f"))
            for e in range(EC):
                nc.sync.dma_start(hT[:, e], hidT[e])
            nc.sync.dma_start(wk_s[:], wkT.rearrange("c p f -> p c f"))
            nc.sync.dma_start(wv_s[:], wvT.rearrange("c p f -> p c f"))
            nc.sync.dma_start(wo_s[:], woT.rearrange("c p f -> p c f"))

            qT = qkv.tile([P, FC, S], BF16, tag="qT")
            kT = qkv.tile([P, FC, S], BF16, tag="kT")
            # vaug[tok, kc, h, 0:64] = V, [..., 64] = 1 (denominator col)
            vaug = qkv.tile([P, KC, H, 65], BF16, tag="vaug")
            nc.gpsimd.memset(vaug[:, :, :, 64:65], 1.0)
            ctxn = qkv.tile([P, FC, S], BF16, tag="ctxn")

            # ---- Q / K projections: out [feat 128p, tok 1024] ----
            def project_qk(w_s, b_t, dst, scale):
                for fc in range(FC):
                    for hb in range(2):
                        pp = psA.tile([P, 1024], F32, tag="A")
                        for sec in range(2):
                            t0 = hb * 1024 + sec * 512
                            for e in range(EC):
                                nc.tensor.matmul(
                                    pp[:, bass.ts(sec, 512)],
                                    w_s[:, e, bass.ts(fc, P)],
                                    hT[:, e, t0:t0 + 512],
                                    start=(e == 0), stop=(e == EC - 1))
                        if scale is None:
                            nc.vector.tensor_scalar_add(
                                dst[:, fc, bass.ts(hb, 1024)], pp[:],
                                b_t[:, fc:fc + 1])
                        else:
                            nc.vector.tensor_scalar(
                                dst[:, fc, bass.ts(hb, 1024)], pp[:],
                                b_t[:, fc:fc + 1], scale,
                                mybir.AluOpType.add, mybir.AluOpType.mult)

            project_qk(wq_s, bq_t, qT, 0.125)
            project_qk(wk_s, bk_t, kT, None)

            # ---- V projection for a pair of token chunks ----
            def project_v(pair):
                pp = psA.tile([P, 1024], F32, tag="A")
                for j in range(2):
                    tc0 = (pair * 2 + j) * P
                    for e in range(EC):
                        nc.tensor.matmul(pp[:, bass.ts(j, 512)],
                                         hT[:, e, tc0:tc0 + P],
                                         wv_s[:, e, :],
                                         start=(e == 0), stop=(e == EC - 1))
                nc.vector.tensor_tensor(
                    vaug[:, pair * 2:pair * 2 + 2, :, 0:64],
                    pp[:].rearrange("p (c h f) -> p c h f", c=2, h=H),
                    bv_b[:, None, :, :].to_broadcast((P, 2, H, 64)),
                    mybir.AluOpType.add)

            # ---- attention unit (head h, q block qb) ----
            # exp runs on ACT except OFFLOAD chunks, which use a Schraudolph
            # bf16-bits exp on DVE (int16 round of y*128 + bias, bitcast)
            def attention(h, qb, interleave_v):
                fc, hh = h >> 1, (h & 1) * 64
                q0 = qb * 1024
                psc = psC.tile([65, 1024], F32)
                prev = None
                for c in range(KC):
                    if interleave_v and c % 2 == 0:
                        project_v(c // 2)
                    pss = psA.tile([P, 1024], F32, tag="A")
                    for qt in range(2):
                        nc.tensor.matmul(
                            pss[:, bass.ts(qt, 512)],
                            kT[hh:hh + 64, fc, bass.ts(c, P)],
                            qT[hh:hh + 64, fc, q0 + qt * 512:q0 + (qt + 1) * 512],
                            start=True, stop=True)
                    if c in OFFLOAD:
                        pri = probsp.tile([P, 1024], I16, tag="pri")
                        nc.vector.tensor_scalar(
                            pri[:], pss[:], SCH_MUL, SCH_BIAS,
                            mybir.AluOpType.mult, mybir.AluOpType.add)
                        pr = pri[:].bitcast(BF16)
                    else:
                        prt = probsp.tile([P, 1024], BF16, tag="pr")
                        nc.scalar.activation(prt[:], pss[:], EXP)
                        pr = prt[:]
                    # ctx matmuls for the PREVIOUS chunk (sw pipeline keeps
                    # PE from stalling on the current exp)
                    if prev is not None:
                        pv, cc = prev
                        for qt in range(2):
                            nc.tensor.matmul(
                                psc[:, bass.ts(qt, 512)],
                                vaug[:, cc, h, :],
                                pv[:, qt * 512:(qt + 1) * 512],
                                start=(cc == 0), stop=False)
                    prev = (pr, c)
                pv, cc = prev
                for qt in range(2):
                    nc.tensor.matmul(psc[:, bass.ts(qt, 512)],
                                     vaug[:, cc, h, :],
                                     pv[:, qt * 512:(qt + 1) * 512],
                                     start=False, stop=True)

                # normalize: copy denom row to sbuf p64, broadcast it to
                # partitions 0..63 via a 1-contract PE matmul (ones-col x
                # den-row), recip psum->sbuf at offset 0, multiply
                dsb = recipp.tile([65, 1024], F32R, tag="dsb")
                nc.vector.tensor_copy(dsb[64:65, :], psc[64:65, :])
                rbp = psA.tile([P, 1024], F32, tag="A")
                for qt in range(2):
                    nc.tensor.matmul(rbp[0:64, bass.ts(qt, 512)],
                                     ones_r[64:65, :],
                                     dsb[64:65, qt * 512:(qt + 1) * 512],
                                     start=True, stop=True)
                rbs = recipbp.tile([64, 1024], F32)
                nc.vector.reciprocal_approx_fast(rbs[:], rbp[0:64, :])
                nc.vector.tensor_tensor(
                    ctxn[hh:hh + 64, fc, q0:q0 + 1024],
                    psc[0:64, :], rbs[:], mybir.AluOpType.mult)

            # ---- partial output projection for one q block ----
            def out_proj(qb):
                for t in range(8):
                    tc0 = qb * 1024 + t * P
                    po = psA.tile([P, 1024], F32, tag="A")
                    for sec in range(2):
                        for cc in range(FC):
                            nc.tensor.matmul(
                                po[:, bass.ts(sec, 512)],
                                ctxn[:, cc, tc0:tc0 + P],
                                wo_s[:, cc, bass.ts(sec, 512)],
                                start=(cc == 0), stop=(cc == FC - 1))
                    ot = ostage.tile([P, 1024], BF16)
                    nc.vector.tensor_copy(ot[:], po[:])
                    nc.sync.dma_start(out[tc0:tc0 + P, :], ot[:])

            for qb in range(QB):
                for h in range(H):
                    attention(h, qb, interleave_v=(qb == 0 and h == 0))
                out_proj(qb)

    nc.compile()
    return nc


_NC_CACHE = None


def build_in_maps(hid, Wq, bq, Wk, bk, Wv, bv, Wo):
    hid = np.asarray(hid, np.float32)
    Wq = np.asarray(Wq, np.float32)
    Wk = np.asarray(Wk, np.float32)
    Wv = np.asarray(Wv, np.float32)
    Wo = np.asarray(Wo, np.float32)
    bq = np.asarray(bq, np.float32)
    bk = np.asarray(bk, np.float32)
    bv = np.asarray(bv, np.float32)

    in_maps = []
    for c in range(NCORES):
        b, g = divmod(c, 2)
        fs = slice(g * F, (g + 1) * F)
        hT = np.ascontiguousarray(hid[b].T).astype(BF).reshape(EC, P, S)
        in_maps.append({
            "hidT": hT,
            "wqT": np.ascontiguousarray(Wq[fs].T).astype(BF).reshape(EC, P, F),
            "wkT": np.ascontiguousarray(Wk[fs].T).astype(BF).reshape(EC, P, F),
            "wvT": np.ascontiguousarray(Wv[fs].T).astype(BF).reshape(EC, P, F),
            "woT": np.ascontiguousarray(Wo[:, fs].T).astype(BF).reshape(FC, P, D),
            "bq": np.ascontiguousarray(bq[fs].reshape(FC, P).T),
            "bk": np.ascontiguousarray(bk[fs].reshape(FC, P).T),
            "bv": bv[fs].reshape(1, F).copy(),
        })
    return in_maps


def kernel(hidden_states, Wq, bq, Wk, bk, Wv, bv, Wo, bo):
    global _NC_CACHE
    if _NC_CACHE is None:
        _NC_CACHE = build_nc()
    nc = _NC_CACHE

    in_maps = build_in_maps(hidden_states, Wq, bq, Wk, bk, Wv, bv, Wo)

    try:
        res = bass_utils.run_bass_kernel_spmd(nc, in_maps,
                                              core_ids=list(range(NCORES)))
    except Exception:
        # transient device flake (e.g. NRT_EXEC_UNIT_UNRECOVERABLE): retry once
        res = bass_utils.run_bass_kernel_spmd(nc, in_maps,
                                              core_ids=list(range(NCORES)))

    bo = np.asarray(bo, dtype=np.float32)
    outp = np.empty((B, S, D), dtype=np.float32)
    for b in range(B):
        acc = res.results[2 * b]["out"].astype(np.float32)
        acc += res.results[2 * b + 1]["out"].astype(np.float32)
        outp[b] = acc + bo
    return outp


# revision 10
# speedup vs baseline: 1.4241x; 1.4241x over previous
"""Multi-head attention Trainium2 Bass kernel (v2: bf16 + DP4xTP2).

Problem: B=4, S=2048, H=16, DH=64, D=1024, fp32 in/out.
  q/k/v = hidden @ W{q,k,v}.T + b; scores = q k^T / 8; probs = softmax;
  ctx = probs v; out = ctx @ Wo.T + bo.

Sharding: 4-way data-parallel over batch x 2-way tensor-parallel over
heads. Core c owns batch c//2 and heads (c%2)*8 .. +8 (a 512-wide
feature slice). Each core computes a partial output projection over its
512 context features; host sums the 2 partials per batch and adds bo.

All matmuls in bf16 (fp32r streams rows at ~2 cyc on HW, bf16 at 1).
Host pre-transposes hidden and weights so the device does ZERO PE
transposes:
  hT   [e(8x128p), tok 2048]   <- host hid[b].T
  qT/kT [feat(4x128p), tok]    = WT-lhsT x hT-rhs   (+bias, q pre-scaled)
  V    [tok(16x128p), feat 512] = hT-lhsT x WvT-rhs  (+bias) -> vaug
  vaug [tok 128p, kc 16, h 8, 65] (64 V cols + ones col for denominator)
  S    [kt 128p, q 1024] = kT-lhsT x qT-rhs  (PSUM, fp32)
  P    = exp(S) -> bf16 (ACT engine; no max subtraction: scores ~ N(0,1))
  ctx^T/den = vaug-lhsT x P-rhs accumulated over kt chunks (PSUM [65, q])
  ctxn = ctx^T * (1/den)  (recip on DVE, partition-broadcast on GpSimd)
  out_partial [tok, fo] = ctxn-lhsT x woT-rhs -> bf16 -> DRAM
"""
import numpy as np
import ml_dtypes

import concourse.bass as bass
import concourse.tile as tile
from concourse import bacc, mybir
from concourse import bass_utils

F32 = mybir.dt.float32
F32R = mybir.dt.float32r
BF16 = mybir.dt.bfloat16
I16 = mybir.dt.int16
EXP = mybir.ActivationFunctionType.Exp
BF = ml_dtypes.bfloat16

# Schraudolph exp in bf16 bit-space: bits = round(x*log2e*128 + 127*128 - 7)
SCH_MUL = 128.0 * 1.4426950408889634
SCH_BIAS = 127.0 * 128 - 7.0
OFFLOAD = {2, 5, 8, 11, 14}   # kt chunks whose exp runs on DVE instead of ACT

B = 4
S = 2048
D = 1024
NCORES = 8
P = 128          # partitions
F = 512          # per-core feature slice (8 heads x 64)
H = 8            # heads per core
EC = D // P      # 8 e-chunks (contraction for projections)
FC = F // P      # 4 feature chunks
KC = S // P      # 16 kt chunks
QB = S // 1024   # 2 q blocks


def build_nc():
    nc = bacc.Bacc("TRN2", target_bir_lowering=False, debug=False,
                   enable_asserts=True, num_devices=NCORES)

    hidT = nc.dram_tensor("hidT", [EC, P, S], BF16, kind="ExternalInput").ap()
    wqT = nc.dram_tensor("wqT", [EC, P, F], BF16, kind="ExternalInput").ap()
    wkT = nc.dram_tensor("wkT", [EC, P, F], BF16, kind="ExternalInput").ap()
    wvT = nc.dram_tensor("wvT", [EC, P, F], BF16, kind="ExternalInput").ap()
    woT = nc.dram_tensor("woT", [FC, P, D], BF16, kind="ExternalInput").ap()
    bq = nc.dram_tensor("bq", [P, FC], F32, kind="ExternalInput").ap()
    bk = nc.dram_tensor("bk", [P, FC], F32, kind="ExternalInput").ap()
    bv = nc.dram_tensor("bv", [1, F], F32, kind="ExternalInput").ap()
    out = nc.dram_tensor("out", [S, D], BF16, kind="ExternalOutput").ap()

    with tile.TileContext(nc) as tc:
        with (
            tc.tile_pool(name="const", bufs=1) as cpool,
            tc.tile_pool(name="wts", bufs=1) as wpool,
            tc.tile_pool(name="qkv", bufs=1) as qkv,
            tc.tile_pool(name="probs", bufs=4) as probsp,
            tc.tile_pool(name="recipp", bufs=2) as recipp,
            tc.tile_pool(name="recipbp", bufs=2) as recipbp,
            tc.tile_pool(name="ostage", bufs=4) as ostage,
            tc.tile_pool(name="psA", bufs=2, space="PSUM") as psA,
            tc.tile_pool(name="psC", bufs=2, space="PSUM") as psC,
        ):
            # ---- constants / weights ----
            bq_t = cpool.tile([P, FC], F32, tag="bq")
            bk_t = cpool.tile([P, FC], F32, tag="bk")
            bv_1 = cpool.tile([1, F], F32, tag="bv1")
            nc.sync.dma_start(bq_t[:], bq)
            nc.sync.dma_start(bk_t[:], bk)
            nc.sync.dma_start(bv_1[:], bv)
            # bv broadcast across partitions, viewed [P, h, 64]
            bv_b = cpool.tile([P, H, 64], F32, tag="bvb")
            nc.gpsimd.partition_broadcast(
                bv_b[:].rearrange("p h f -> p (h f)"), bv_1[:])
            # ones row (partition 64) for the denominator-broadcast matmul
            ones32 = cpool.tile([65, 64], F32, tag="ones32")
            nc.gpsimd.memset(ones32[:], 1.0)
            ones_r = cpool.tile([65, 64], F32R, tag="onesr")
            nc.vector.tensor_copy(ones_r[:], ones32[:])

            wq_s = wpool.tile([P, EC, F], BF16, tag="wq")
            wk_s = wpool.tile([P, EC, F], BF16, tag="wk")
            wv_s = wpool.tile([P, EC, F], BF16, tag="wv")
            wo_s = wpool.tile([P, FC, D], BF16, tag="wo")
            hT = wpool.tile([P, EC, S], BF16, tag="hT")
            nc.sync.dma_start(wq_s[:], wqT.rearrange("c p f -> p c f"))
            for e in range(EC):
                nc.sync.dma_start(hT[:, e], hidT[e])
            nc.sync.dma_start(wk_s[:], wkT.rearrange("c p f -> p c f"))
            nc.sync.dma_start(wv_s[:], wvT.rearrange("c p f -> p c f"))
            nc.sync.dma_start(wo_s[:], woT.rearrange("c p f -> p c f"))

            qT = qkv.tile([P, FC, S], BF16, tag="qT")
            kT = qkv.tile([P, FC, S], BF16, tag="kT")
            # vaug[tok, kc, h, 0:64] = V, [..., 64] = 1 (denominator col)
            vaug = qkv.tile([P, KC, H, 65], BF16, tag="vaug")
            nc.gpsimd.memset(vaug[:, :, :, 64:65], 1.0)
            ctxn = qkv.tile([P, FC, S], BF16, tag="ctxn")

            # ---- Q / K projections: out [feat 128p, tok 1024] ----
            def project_qk(w_s, b_t, dst, scale):
                for fc in range(FC):
                    for hb in range(2):
                        pp = psA.tile([P, 1024], F32, tag="A")
                        for sec in range(2):
                            t0 = hb * 1024 + sec * 512
                            for e in range(EC):
                                nc.tensor.matmul(
                                    pp[:, bass.ts(sec, 512)],
                                    w_s[:, e, bass.ts(fc, P)],
                                    hT[:, e, t0:t0 + 512],
                                    start=(e == 0), stop=(e == EC - 1))
                        if scale is None:
                            nc.vector.tensor_scalar_add(
                                dst[:, fc, bass.ts(hb, 1024)], pp[:],
                                b_t[:, fc:fc + 1])
                        else:
                            nc.vector.tensor_scalar(
                                dst[:, fc, bass.ts(hb, 1024)], pp[:],
                                b_t[:, fc:fc + 1], scale,
                                mybir.AluOpType.add, mybir.AluOpType.mult)

            project_qk(wq_s, bq_t, qT, 0.125)
            project_qk(wk_s, bk_t, kT, None)

            # ---- V projection for a pair of token chunks ----
            def project_v(pair):
                pp = psA.tile([P, 1024], F32, tag="A")
                for j in range(2):
                    tc0 = (pair * 2 + j) * P
                    for e in range(EC):
                        nc.tensor.matmul(pp[:, bass.ts(j, 512)],
                                         hT[:, e, tc0:tc0 + P],
                                         wv_s[:, e, :],
                                         start=(e == 0), stop=(e == EC - 1))
                nc.vector.tensor_tensor(
                    vaug[:, pair * 2:pair * 2 + 2, :, 0:64],
                    pp[:].rearrange("p (c h f) -> p c h f", c=2, h=H),
                    bv_b[:, None, :, :].to_broadcast((P, 2, H, 64)),
                    mybir.AluOpType.add)

            # ---- deferred normalize tail: broadcast the denom to
            # partitions 0..63 via a 1-contract PE matmul (ones-col x
            # den-row), recip psum->sbuf at offset 0, multiply.
            # Emitted INSIDE the next unit's chunk stream so the in-order
            # PE queue never waits on the DVE denom copy.
            def norm_tail(st):
                h, qb, psc, dsb = st
                fc, hh = h >> 1, (h & 1) * 64
                q0 = qb * 1024
                rbp = psA.tile([P, 1024], F32, tag="A")
                for qt in range(2):
                    nc.tensor.matmul(rbp[0:64, bass.ts(qt, 512)],
                                     ones_r[64:65, :],
                                     dsb[64:65, qt * 512:(qt + 1) * 512],
                                     start=True, stop=True)
                rbs = recipbp.tile([64, 1024], F32)
                nc.vector.reciprocal_approx_fast(rbs[:], rbp[0:64, :])
                nc.vector.tensor_tensor(
                    ctxn[hh:hh + 64, fc, q0:q0 + 1024],
                    psc[0:64, :], rbs[:], mybir.AluOpType.mult)

            pending = []

            # ---- attention unit (head h, q block qb) ----
            # exp runs on ACT except OFFLOAD chunks, which use a Schraudolph
            # bf16-bits exp on DVE (int16 round of y*128 + bias, bitcast)
            def attention(h, qb, interleave_v):
                fc, hh = h >> 1, (h & 1) * 64
                q0 = qb * 1024
                psc = psC.tile([65, 1024], F32)
                prev = None
                for c in range(KC):
                    if interleave_v and c % 2 == 0:
                        project_v(c // 2)
                    pss = psA.tile([P, 1024], F32, tag="A")
                    for qt in range(2):
                        nc.tensor.matmul(
                            pss[:, bass.ts(qt, 512)],
                            kT[hh:hh + 64, fc, bass.ts(c, P)],
                            qT[hh:hh + 64, fc, q0 + qt * 512:q0 + (qt + 1) * 512],
                            start=True, stop=True)
                    if c in OFFLOAD:
                        pri = probsp.tile([P, 1024], I16, tag="pri")
                        nc.vector.tensor_scalar(
                            pri[:], pss[:], SCH_MUL, SCH_BIAS,
                            mybir.AluOpType.mult, mybir.AluOpType.add)
                        pr = pri[:].bitcast(BF16)
                    else:
                        prt = probsp.tile([P, 1024], BF16, tag="pr")
                        nc.scalar.activation(prt[:], pss[:], EXP)
                        pr = prt[:]
                    if c == 3 and pending:
                        norm_tail(pending.pop())
                    # ctx matmuls for the PREVIOUS chunk (sw pipeline keeps
                    # PE from stalling on the current exp)
                    if prev is not None:
                        pv, cc = prev
                        for qt in range(2):
                            nc.tensor.matmul(
                                psc[:, bass.ts(qt, 512)],
                                vaug[:, cc, h, :],
                                pv[:, qt * 512:(qt + 1) * 512],
                                start=(cc == 0), stop=False)
                    prev = (pr, c)
                pv, cc = prev
                for qt in range(2):
                    nc.tensor.matmul(psc[:, bass.ts(qt, 512)],
                                     vaug[:, cc, h, :],
                                     pv[:, qt * 512:(qt + 1) * 512],
                                     start=False, stop=True)

                # denom row psum p64 -> sbuf p64 now (DVE); rest deferred
                dsb = recipp.tile([65, 1024], F32R, tag="dsb")
                nc.vector.tensor_copy(dsb[64:65, :], psc[64:65, :])
                pending.append((h, qb, psc, dsb))

            # ---- partial output projection for one q block ----
            def out_proj(qb):
                for t in range(8):
                    tc0 = qb * 1024 + t * P
                    po = psA.tile([P, 1024], F32, tag="A")
                    for sec in range(2):
                        for cc in range(FC):
                            nc.tensor.matmul(
                                po[:, bass.ts(sec, 512)],
                                ctxn[:, cc, tc0:tc0 + P],
                                wo_s[:, cc, bass.ts(sec, 512)],
                                start=(cc == 0), stop=(cc == FC - 1))
                    ot = ostage.tile([P, 1024], BF16)
                    nc.vector.tensor_copy(ot[:], po[:])
                    nc.sync.dma_start(out[tc0:tc0 + P, :], ot[:])

            for qb in range(QB):
                for h in range(H):
                    attention(h, qb, interleave_v=(qb == 0 and h == 0))
                while pending:
                    norm_tail(pending.pop())
                out_proj(qb)

    nc.compile()
    return nc


_NC_CACHE = None


def build_in_maps(hid, Wq, bq, Wk, bk, Wv, bv, Wo):
    hid = np.asarray(hid, np.float32)
    Wq = np.asarray(Wq, np.float32)
    Wk = np.asarray(Wk, np.float32)
    Wv = np.asarray(Wv, np.float32)
    Wo = np.asarray(Wo, np.float32)
    bq = np.asarray(bq, np.float32)
    bk = np.asarray(bk, np.float32)
    bv = np.asarray(bv, np.float32)

    in_maps = []
    for c in range(NCORES):
        b, g = divmod(c, 2)
        fs = slice(g * F, (g + 1) * F)
        hT = np.ascontiguousarray(hid[b].T).astype(BF).reshape(EC, P, S)
        in_maps.append({
            "hidT": hT,
            "wqT": np.ascontiguousarray(Wq[fs].T).astype(BF).reshape(EC, P, F),
            "wkT": np.ascontiguousarray(Wk[fs].T).astype(BF).reshape(EC, P, F),
            "wvT": np.ascontiguousarray(Wv[fs].T).astype(BF).reshape(EC, P, F),
            "woT": np.ascontiguousarray(Wo[:, fs].T).astype(BF).reshape(FC, P, D),
            "bq": np.ascontiguousarray(bq[fs].reshape(FC, P).T),
            "bk": np.ascontiguousarray(bk[fs].reshape(FC, P).T),
            "bv": bv[fs].reshape(1, F).copy(),
        })
    return in_maps


def kernel(hidden_states, Wq, bq, Wk, bk, Wv, bv, Wo, bo):
    global _NC_CACHE
    if _NC_CACHE is None:
        _NC_CACHE = build_nc()
    nc = _NC_CACHE

    in_maps = build_in_maps(hidden_states, Wq, bq, Wk, bk, Wv, bv, Wo)

    try:
        res = bass_utils.run_bass_kernel_spmd(nc, in_maps,
                                              core_ids=list(range(NCORES)))
    except Exception:
        # transient device flake (e.g. NRT_EXEC_UNIT_UNRECOVERABLE): retry once
        res = bass_utils.run_bass_kernel_spmd(nc, in_maps,
                                              core_ids=list(range(NCORES)))

    bo = np.asarray(bo, dtype=np.float32)
    outp = np.empty((B, S, D), dtype=np.float32)
    for b in range(B):
        acc = res.results[2 * b]["out"].astype(np.float32)
        acc += res.results[2 * b + 1]["out"].astype(np.float32)
        outp[b] = acc + bo
    return outp
